# revision 2
# baseline (speedup 1.0000x reference)
# MoE (8 experts, top-2, SwiGLU) Trainium2 kernel.
#
# Expert-parallel: core e owns expert e; host routes/gathers tokens, device
# does the dense SwiGLU FFN in bf16 with fp32 PSUM accumulation, output is
# scaled by routing weight on-device; host scatter-adds.
#
# Startup is HBM-supply-bound (the PE needs w1/w2 of m=0 plus all 8 xg
# chunks, ~2.7MB, before it can stream): both HW DGE queues (Sync+Scalar)
# carry the startup transfers in exact consumption order, phase-1 m=0 runs
# k-outer with 6 concurrent PSUM chains so each arriving xg chunk feeds
# ~0.9us of PE work, and all phase-2-only inputs (w3, wt) plus the w12
# prefetch are pushed out of the startup window with scheduler wait hints.
# Output is stored bf16 (well within the error budget) to shorten the tail.
#
# Shapes (hardcoded): x [2, 2048, 1024] f32, gate_w [8, 1024],
# w1/w2 [8, 2730, 1024], w3 [8, 1024, 2730]. N=4096, C=1024, H=2730 (pad 2816).

import numpy as np
import ml_dtypes

NUM_EXPERTS = 8
TOP_K = 2
C = 1024
H = 2730
H2 = 2816  # H padded to a multiple of 128 (zero rows contribute nothing)
KC = C // 128  # 8 contraction chunks over C
MH = H2 // 128  # 22 chunks over padded H
N_CORES = 8

_bf16 = ml_dtypes.bfloat16

_program_cache: dict[int, object] = {}


def _route_host(xt: np.ndarray, gate_w: np.ndarray):
    """Mirror of the reference router in fp32 numpy."""
    logits = xt @ gate_w.T.astype(np.float32)  # [N, E] fp32
    i1 = np.argmax(logits, axis=1)
    n_idx = np.arange(logits.shape[0])
    v1 = logits[n_idx, i1]
    masked = logits.copy()
    masked[n_idx, i1] = -np.inf
    i2 = np.argmax(masked, axis=1)
    v2 = masked[n_idx, i2]
    e2 = np.exp((v2 - v1).astype(np.float32))
    w1 = (1.0 / (1.0 + e2)).astype(np.float32)
    w2 = (e2 / (1.0 + e2)).astype(np.float32)
    top_idx = np.stack([i1, i2], axis=1)
    top_w = np.stack([w1, w2], axis=1)
    return top_idx, top_w


def _token_tiles(cap: int):
    # near-equal tiles <=512: keeps every matmul's moving dim large enough
    # that the ~100ns LDWEIGHTS always hides under the matmul
    nsplit = max(1, (cap + 511) // 512)
    tiles = []
    n0 = 0
    for i in range(nsplit):
        nw = ((cap - n0) + (nsplit - 1 - i)) // (nsplit - i)
        nw = (nw + 3) // 4 * 4
        nw = min(nw, cap - n0)
        tiles.append((n0, nw))
        n0 += nw
    return tiles


def _build_program(cap: int):
    """Build the SPMD Bass program for per-core token capacity `cap`."""
    import concourse.bass as bass
    import concourse.mybir as mybir
    from concourse import bacc
    from concourse.tile import TileContext

    dt = mybir.dt
    tiles = _token_tiles(cap)

    nc = bacc.Bacc(None, target_bir_lowering=False)
    xgT_d = nc.declare_dram_parameter("xgT", [KC, 128, cap], dt.bfloat16, isOutput=False)
    # m=0 weights split so the first chains start as soon as w1 lands
    w1a_d = nc.declare_dram_parameter("w1a", [128, KC, 128], dt.bfloat16, isOutput=False)
    w2a_d = nc.declare_dram_parameter("w2a", [128, KC, 128], dt.bfloat16, isOutput=False)
    # combined w1+w2 stream for m >= 1: [m-1, 128, {w1,w2}, KC, 128]
    w12_d = nc.declare_dram_parameter("w12", [MH - 1, 128, 2, KC, 128], dt.bfloat16, isOutput=False)
    w3T_d = nc.declare_dram_parameter("w3T", [MH, 128, C], dt.bfloat16, isOutput=False)
    wtb_d = nc.declare_dram_parameter("wtb", [128, cap], dt.float32, isOutput=False)
    out_d = nc.declare_dram_parameter("out", [C // 128, 128, cap], dt.bfloat16, isOutput=True)

    with TileContext(nc) as tc:
        with (
            tc.tile_pool(name="big", bufs=1) as big,
            tc.tile_pool(name="wstream", bufs=3) as wpool,
            tc.tile_pool(name="work", bufs=3) as work,
            tc.tile_pool(name="psum", bufs=2, space="PSUM") as psum,
        ):
            # Resident SBUF tensors
            xg_k = [big.tile([128, cap], dt.bfloat16, name=f"xg{k}") for k in range(KC)]
            act_sb = big.tile([128, MH, cap], dt.bfloat16)
            w3_sb = big.tile([128, MH, C], dt.bfloat16)
            wt_sb = big.tile([128, cap], dt.float32)
            w1a = big.tile([128, KC, 128], dt.bfloat16)
            w2a = big.tile([128, KC, 128], dt.bfloat16)

            # PE warm-up: dependency-free matmuls on a zeroed scratch tile so
            # the PE pipeline is hot while the first DMAs land (~1.6us cover).
            warm_in = big.tile([128, 128], dt.bfloat16)
            nc.gpsimd.memset(warm_in[:], 0)
            warm_ps = psum.tile([128, 128], dt.float32, tag="ps1", bufs=3,
                                padded_shape=[128, 512], name="warm_ps")
            for _ in range(32):
                nc.tensor.matmul(warm_ps[:, :64], lhsT=warm_in[:], rhs=warm_in[:, :64])

            # ---- startup DMAs: whole chunks alternating across the two HW
            # DGE queues in consumption order — two active queues pull more
            # aggregate bandwidth than one, without extra descriptors ----
            nc.sync.dma_start(out=w1a[:], in_=w1a_d[:])
            nc.scalar.dma_start(out=xg_k[0][:], in_=xgT_d[0])
            nc.sync.dma_start(out=xg_k[1][:], in_=xgT_d[1])
            nc.scalar.dma_start(out=w2a[:], in_=w2a_d[:])
            for k in range(2, KC):
                eng = nc.sync if k % 2 == 0 else nc.scalar
                eng.dma_start(out=xg_k[k][:], in_=xgT_d[k])

            # wt + w3 are phase-2-only inputs: push them out of the
            # bandwidth-critical startup window with scheduler wait hints
            with tc.tile_wait_until(0.040):
                nc.scalar.dma_start(out=wt_sb[:], in_=wtb_d[:])

            # ---- Phase 1, m=0: k-outer with 3 concurrent PSUM chains so the
            # matmuls consume xg_k[k] progressively as the DMAs land ----
            with tc.tile_wait_until(0.030):
                nc.scalar.dma_start(out=w3_sb[:, 0, :], in_=w3T_d[0])
            # both w1 and w2 chains run k-outer simultaneously (6 live PSUM
            # chains) so every arriving xg chunk feeds ~0.9us of PE work —
            # matching the chunk supply cadence of the DMA fabric
            ps_a = [
                psum.tile([128, nw], dt.float32, tag="ps1", bufs=3,
                          padded_shape=[128, 512], name=f"p0a_{ti}")
                for ti, (n0, nw) in enumerate(tiles)
            ]
            ps_b = [
                psum.tile([128, nw], dt.float32, tag="ps2", bufs=3,
                          padded_shape=[128, 512], name=f"p0b_{ti}")
                for ti, (n0, nw) in enumerate(tiles)
            ]
            for k in range(KC):
                for ti, (n0, nw) in enumerate(tiles):
                    nc.tensor.matmul(
                        ps_a[ti][:], lhsT=w1a[:, k, :], rhs=xg_k[k][:, n0:n0 + nw],
                        start=(k == 0), stop=(k == KC - 1),
                    )
                for ti, (n0, nw) in enumerate(tiles):
                    nc.tensor.matmul(
                        ps_b[ti][:], lhsT=w2a[:, k, :], rhs=xg_k[k][:, n0:n0 + nw],
                        start=(k == 0), stop=(k == KC - 1),
                    )
            for ti, (n0, nw) in enumerate(tiles):
                tmp = work.tile([128, nw], dt.bfloat16, tag="tmp",
                                padded_shape=[128, 512], name=f"tmp0_{ti}")
                nc.scalar.activation(tmp[:], ps_a[ti][:], mybir.ActivationFunctionType.Silu)
                nc.vector.tensor_mul(act_sb[:, 0, n0:n0 + nw], tmp[:], ps_b[ti][:])

            # ---- Phase 1, m>=1: tile-outer, combined w12 stream ----
            for m in range(1, MH):
                w12 = wpool.tile([128, 2, KC, 128], dt.bfloat16, tag="w12")
                # the first few w12 prefetches would otherwise issue during
                # the startup window and steal HBM bandwidth from xg
                with tc.tile_wait_until(0.014 + 0.007 * (m - 1), enable=(m <= 3)):
                    nc.sync.dma_start(out=w12[:], in_=w12_d[m - 1])
                # w3 is phase-2-only: spread the loads over the phase-1 body
                with tc.tile_wait_until(0.030 + 0.005 * m):
                    nc.scalar.dma_start(out=w3_sb[:, m, :], in_=w3T_d[m])

                for (n0, nw) in tiles:
                    ps1 = psum.tile([128, nw], dt.float32, tag="ps1", bufs=3,
                                    padded_shape=[128, 512])
                    ps2 = psum.tile([128, nw], dt.float32, tag="ps2", bufs=3,
                                    padded_shape=[128, 512])
                    for k in range(KC):
                        nc.tensor.matmul(
                            ps1[:], lhsT=w12[:, 0, k, :], rhs=xg_k[k][:, n0:n0 + nw],
                            start=(k == 0), stop=(k == KC - 1),
                        )
                    for k in range(KC):
                        nc.tensor.matmul(
                            ps2[:], lhsT=w12[:, 1, k, :], rhs=xg_k[k][:, n0:n0 + nw],
                            start=(k == 0), stop=(k == KC - 1),
                        )
                    tmp = work.tile([128, nw], dt.bfloat16, tag="tmp", padded_shape=[128, 512])
                    nc.scalar.activation(tmp[:], ps1[:], mybir.ActivationFunctionType.Silu)
                    nc.vector.tensor_mul(act_sb[:, m, n0:n0 + nw], tmp[:], ps2[:])

            # ---- Phase 2: out[c_out, tok] = sum_m w3[m]^T @ act[m] ----
            for (n0, nw) in tiles:
                for co in range(C // 128):
                    ps3 = psum.tile([128, nw], dt.float32, tag="ps3", bufs=2,
                                    padded_shape=[128, 512])
                    for m in range(MH):
                        nc.tensor.matmul(
                            ps3[:],
                            lhsT=w3_sb[:, m, co * 128:(co + 1) * 128],
                            rhs=act_sb[:, m, n0:n0 + nw],
                            start=(m == 0), stop=(m == MH - 1),
                        )
                    o_sb = work.tile([128, nw], dt.bfloat16, tag="osb", bufs=4,
                                     padded_shape=[128, 512])
                    nc.vector.tensor_mul(o_sb[:], ps3[:], wt_sb[:, n0:n0 + nw])
                    nc.sync.dma_start(out=out_d[co][:, n0:n0 + nw], in_=o_sb[:])

    nc.finalize()
    return nc


def _prepare_core_inputs(xt, w1, w2, w3, top_idx, top_w):
    """Host-side dispatch: gather tokens per expert, pad, transpose, cast."""
    idx_lists = []
    wt_lists = []
    for e in range(NUM_EXPERTS):
        m0 = top_idx[:, 0] == e
        m1 = top_idx[:, 1] == e
        sel = m0 | m1
        idx_e = np.nonzero(sel)[0]
        wt_e = np.where(m0[idx_e], top_w[idx_e, 0], top_w[idx_e, 1]).astype(np.float32)
        idx_lists.append(idx_e)
        wt_lists.append(wt_e)

    max_cnt = max(len(i) for i in idx_lists)
    cap = max(128, ((max_cnt + 3) // 4) * 4)  # 4-aligned for clean APs

    in_maps = []
    for e in range(NUM_EXPERTS):
        idx_e = idx_lists[e]
        cnt = len(idx_e)
        xg = np.zeros((cap, C), np.float32)
        xg[:cnt] = xt[idx_e]
        xgT = np.ascontiguousarray(xg.T.reshape(KC, 128, cap)).astype(_bf16)

        w1p = np.zeros((H2, C), np.float32)
        w1p[:H] = w1[e]
        w2p = np.zeros((H2, C), np.float32)
        w2p[:H] = w2[e]
        w3p = np.zeros((C, H2), np.float32)
        w3p[:, :H] = w3[e]

        # [MH, 128(part=c within chunk), KC, 128(h within chunk)]
        w1T = np.ascontiguousarray(
            w1p.T.reshape(KC, 128, MH, 128).transpose(2, 1, 0, 3)
        ).astype(_bf16)
        w2T = np.ascontiguousarray(
            w2p.T.reshape(KC, 128, MH, 128).transpose(2, 1, 0, 3)
        ).astype(_bf16)
        w12 = np.ascontiguousarray(
            np.stack([w1T[1:], w2T[1:]], axis=2)  # [MH-1, 128, 2, KC, 128]
        )
        # [MH, 128(part=h within chunk), C]
        w3T = np.ascontiguousarray(w3p.T.reshape(MH, 128, C)).astype(_bf16)

        wt_pad = np.zeros(cap, np.float32)
        wt_pad[:cnt] = wt_lists[e]
        wtb = np.ascontiguousarray(np.broadcast_to(wt_pad[None, :], (128, cap)))

        in_maps.append({
            "xgT": xgT, "w1a": w1T[0], "w2a": w2T[0], "w12": w12,
            "w3T": w3T, "wtb": wtb,
        })
    return in_maps, idx_lists, cap


def _run(x, gate_w, w1, w2, w3, trace=False):
    from concourse.bass_utils import run_bass_kernel_spmd

    x = np.asarray(x, dtype=np.float32)
    gate_w = np.asarray(gate_w, dtype=np.float32)
    w1 = np.asarray(w1, dtype=np.float32)
    w2 = np.asarray(w2, dtype=np.float32)
    w3 = np.asarray(w3, dtype=np.float32)

    B, T, Cx = x.shape
    assert Cx == C
    xt = x.reshape(-1, C)

    top_idx, top_w = _route_host(xt, gate_w)
    in_maps, idx_lists, cap = _prepare_core_inputs(xt, w1, w2, w3, top_idx, top_w)

    if cap not in _program_cache:
        _program_cache[cap] = _build_program(cap)
    nc = _program_cache[cap]

    res = run_bass_kernel_spmd(nc, in_maps, list(range(N_CORES)), trace=trace)

    out = np.zeros((xt.shape[0], C), np.float32)
    for e in range(NUM_EXPERTS):
        idx_e = idx_lists[e]
        cnt = len(idx_e)
        # device output is [C, cap] bf16 (c_out on partitions); transpose back
        oe = np.asarray(res.results[e]["out"]).reshape(C, -1).astype(np.float32)
        out[idx_e] += oe[:, :cnt].T

    return out.reshape(B, T, C), res


def kernel(x, gate_w, w1, w2, w3):
    out, _ = _run(x, gate_w, w1, w2, w3, trace=False)
    return out
